# revision 43
# baseline (speedup 1.0000x reference)
"""Chamfer-KL loss kernel for Trainium2 (Bass/Tile).

Math: KL(N_i || N_j) summed over d for all pairs reduces to a rank-10
inner product.  With a = preds, b = gts, d = 4:

  KL[i,j] = 0.5 * (F_i . G_j)
  F_i = [exp(la_i)+mu_a_i^2 (4), -2*mu_a_i (4), 1, -sum_d la_i]
  G_j = [exp(-lb_j) (4), mu_b_j*exp(-lb_j) (4),
         sum_d mu_b_j^2*exp(-lb_j) + sum_d lb_j - 4, 1]

  out = 0.5 * (sum_j min_i (F_i.G_j)  +  sum_i min_j (F_i.G_j))

Sharding: data-parallel over batch, one batch element per NeuronCore
(bs=8 over 8 cores).  Per core the 2048x2048 pairwise matrix is produced
tile-by-tile by the TensorEngine (float32r matmuls, rank 10) into PSUM
and never hits HBM; mins are reduced flash-style on the fly:
  - ScalarE copies each PSUM tile to SBUF as fp16
  - VectorE computes the row-min as a fused fold+reduce over the fp16
    copy and keeps a running elementwise column-min (both lagged one
    tile so they never stall on ScalarE); column mins cross partitions
    at the end via 16 PE transposes + two free-axis reduces.
(GpSimd has no min/max ops and PSUM allows one read operand per
instruction, which rules out the cheaper-looking variants.)
"""

import numpy as np

import concourse.bacc as bacc
import concourse.bass as bass
import concourse.mybir as mybir
import concourse.tile as tile
from concourse.masks import make_identity

BS = 8          # batch size == number of cores
N = 2048        # points per cloud
D = 4           # point dimension
P = 128         # SBUF partitions
PT = N // P     # 16 points per partition in the raw layout
K = 2 * D + 2   # 10 live feature dims
NBLK = 512      # moving-operand columns per matmul (one PSUM bank fp32)
NB = N // NBLK  # 4 j-blocks per i-block
G = N // P      # 16 i-blocks

F32 = mybir.dt.float32
F32R = mybir.dt.float32r
F16 = mybir.dt.float16
AX = mybir.AxisListType.X
OP = mybir.AluOpType
ACTF = mybir.ActivationFunctionType


def _chamfer_tile_kernel(tc, out_dram, mu_a, la, mu_b, lb):
    nc = tc.nc

    sing = tc.alloc_tile_pool(name="sing", bufs=1)
    work = tc.alloc_tile_pool(name="work", bufs=1)
    s_pool = tc.alloc_tile_pool(name="s_pool", bufs=3)

    # Identities first: gpsimd is otherwise idle and the PE pre-warm
    # depends on ident16.
    ident32 = sing.tile([P, P], F32)
    make_identity(nc, ident32)
    ident16 = sing.tile([P, P], F16)
    make_identity(nc, ident16)

    # ---- load raw inputs: [2048, 4] -> [128, 16, 4] (row chunks) ----
    # Three DGE queues; G-side inputs (lb, mu_b) first since the G side
    # gates the first matmuls.
    t_ma = work.tile([P, PT, D], F32)
    t_la = work.tile([P, PT, D], F32)
    t_mb = work.tile([P, PT, D], F32)
    t_lb = work.tile([P, PT, D], F32)
    for (t, src), eng in zip(
            ((t_lb, lb), (t_mb, mu_b), (t_la, la), (t_ma, mu_a)),
            (nc.sync, nc.scalar, nc.gpsimd, nc.sync)):
        eng.dma_start(out=t, in_=src.rearrange("(p t) d -> p t d", p=P))

    # ---- PE pre-warm ----
    # The HAM clock gate keeps a cold PE at half rate for its first
    # ~3.4us; burn no-dep junk matmuls so the feature transposes and the
    # first real matmuls run at full clock.
    with tc.tile_pool(name="warm_psum", bufs=1, space="PSUM") as warm_psum:
        junk = warm_psum.tile([P, P], F32, tag="warm")
        for _ in range(12):
            nc.tensor.matmul(junk, ident16, ident16, start=True, stop=True)

    # ---- feature matrices in interleaved layout [128, 16, 10] ----
    # f128[p, t, k] = feature k of point (16*p + t)
    f128 = work.tile([P, PT, K], F32)
    g128 = work.tile([P, PT, K], F32)

    # G side first: its transposes + copies gate the first matmuls.
    # exp(-lb) is written strided straight into g128 by ACT; DVE reads it
    # back strided, saving two copies.
    nc.scalar.activation(out=g128[:, :, 0:D], in_=t_lb, func=ACTF.Exp,
                         scale=-1.0)
    nc.vector.tensor_mul(g128[:, :, D:2 * D], t_mb, g128[:, :, 0:D])
    t_q2 = work.tile([P, PT, D], F32)
    nc.vector.tensor_mul(t_q2, t_mb, g128[:, :, D:2 * D])
    t_r = work.tile([P, PT], F32)
    nc.vector.tensor_reduce(t_r, t_q2, axis=AX, op=OP.add)
    t_slb = work.tile([P, PT], F32)
    nc.vector.tensor_reduce(t_slb, t_lb, axis=AX, op=OP.add)
    # g128 k=8: (sum_d mub^2 ivb - 4) + sum_d lb, in one fused op
    nc.vector.scalar_tensor_tensor(
        out=g128[:, :, 2 * D], in0=t_r, scalar=-float(D), in1=t_slb,
        op0=OP.add, op1=OP.add)
    nc.vector.memset(g128[:, :, 2 * D + 1], 1.0)

    t_sq = work.tile([P, PT, D], F32)
    nc.vector.tensor_mul(t_sq, t_ma, t_ma)
    nc.scalar.activation(out=f128[:, :, 0:D], in_=t_la, func=ACTF.Exp)
    nc.vector.tensor_tensor(
        f128[:, :, 0:D], f128[:, :, 0:D], t_sq, OP.add)
    nc.vector.tensor_scalar_mul(f128[:, :, D:2 * D], t_ma, -2.0)
    nc.vector.memset(f128[:, :, 2 * D], 1.0)
    nc.vector.tensor_reduce(
        f128[:, :, 2 * D + 1], t_la, axis=AX, op=OP.add, negate=True)

    f128f = f128.rearrange("p t k -> p (t k)")
    g128f = g128.rearrange("p t k -> p (t k)")

    # ---- transpose features so k lands on partitions ----
    # Both sides become [10, 2048] (k on partitions 0..10, all points on
    # the free axis — matmul operands must share base partition 0).
    # G side is split into two [10, 1024] halves so the first matmuls can
    # start after half the copies.
    # Interleave G/F transpose batches with their PSUM->SBUF copies (gt
    # halves on ACT, ft halves on DVE) so copies overlap later transposes.
    with tc.tile_pool(name="pro_psum", bufs=1, space="PSUM") as pro_psum:
        p_gt_a = pro_psum.tile([K, N // 2], F32, tag="gta")
        p_gt_b = pro_psum.tile([K, N // 2], F32, tag="gtb")
        p_ft_a = pro_psum.tile([K, N // 2], F32, tag="fta")
        p_ft_b = pro_psum.tile([K, N // 2], F32, tag="ftb")
        gt_a = work.tile([K, N // 2], F32R)
        gt_b = work.tile([K, N // 2], F32R)
        ft_a = work.tile([K, N // 2], F32R)
        ft_b = work.tile([K, N // 2], F32R)

        def tr_batch(dst, srcf, lo):
            for h in range(lo, lo + 8):
                nc.tensor.transpose(
                    dst[:, P * (h % 8):P * (h % 8 + 1)],
                    srcf[:, K * h:K * (h + 1)], ident32)

        tr_batch(p_gt_a, g128f, 0)
        nc.scalar.copy(gt_a, p_gt_a)
        tr_batch(p_ft_a, f128f, 0)
        nc.vector.tensor_copy(ft_a, p_ft_a)
        tr_batch(p_gt_b, g128f, 8)
        nc.scalar.copy(gt_b, p_gt_b)
        tr_batch(p_ft_b, f128f, 8)
        nc.vector.tensor_copy(ft_b, p_ft_b)

    # ---- main loop: rank-10 matmuls + flash-style min reductions ----
    # (GpSimd supports no min/max ops, so all mins live on VectorE.)
    rm_all = sing.tile([P, G], F32)      # per-i row-min, one column per g
    cm = sing.tile([P, N], F16)          # running column-min

    def rm_update(g, sg):
        # Row-min via two elementwise fold halvings then a short reduce:
        # 2048 -> 1024 -> 512 -> 1 per partition.  Plain TT/reduce ops
        # only (tensor_tensor_reduce dies on hardware).
        f1 = s_pool.tile([P, N // 2], F16, tag="fold", bufs=2, name="f1")
        nc.vector.tensor_tensor(f1, sg[:, 0:N // 2], sg[:, N // 2:N],
                                OP.min)
        f2 = s_pool.tile([P, N // 4], F16, tag="fold2", bufs=2, name="f2")
        nc.vector.tensor_tensor(f2, f1[:, 0:N // 4], f1[:, N // 4:N // 2],
                                OP.min)
        nc.vector.tensor_reduce(
            rm_all[:, g:g + 1], f2, axis=AX, op=OP.min)

    def cm_update(g, sg):
        if g == 0:
            nc.vector.tensor_copy(cm, sg)
        else:
            nc.vector.tensor_tensor(cm, cm, sg, OP.min)

    with tc.tile_pool(name="mm_psum", bufs=2, space="PSUM") as mm_psum:
        sg_prev = None
        for g in range(G):
            pg = mm_psum.tile([P, N], F32, tag="mm")
            ft_t = ft_a if g < 8 else ft_b
            lhsT = ft_t[:, P * (g % 8):P * (g % 8 + 1)]
            for n in range(NB):
                rhs_t = gt_a if n < 2 else gt_b
                nc.tensor.matmul(
                    pg[:, NBLK * n:NBLK * (n + 1)],
                    lhsT,
                    rhs_t[:, NBLK * (n % 2):NBLK * (n % 2 + 1)],
                    start=True, stop=True)
            sg = s_pool.tile([P, N], F16, tag="s", bufs=5)
            nc.scalar.copy(sg, pg)
            # Row-min + column-min both lag one iteration so they consume
            # the previous, already-copied sg — no DVE stall on ACT.  The
            # row-min folds the two sg halves elementwise (throwaway out)
            # and reduces the fold per partition in one fused DVE op.
            if sg_prev is not None:
                rm_update(g - 1, sg_prev)
                cm_update(g - 1, sg_prev)
            sg_prev = sg
        rm_update(G - 1, sg_prev)
        cm_update(G - 1, sg_prev)

    # ---- finalize ----
    # column mins: cross-partition min via 16 PE transposes, then four
    # free-axis reduces over [128, 4, 128] (split so they overlap the
    # transposes).
    with tc.tile_pool(name="fin_psum", bufs=1, space="PSUM") as fin_psum:
        # colmin has G+1 columns: 16 per-chunk column-mins plus the row-min
        # sum folded in as the 17th, so one reduce yields the grand total.
        colmin = sing.tile([P, G + 1], F32)
        nc.vector.tensor_reduce(
            colmin[:, G:G + 1], rm_all, axis=AX, op=OP.add)

        # fin in two tiles so the first reduce starts after 8 transposes
        # (readers of a tile wait on all of its writers).
        fin_a = fin_psum.tile([P, N // 2], F16, tag="fina")
        fin_b = fin_psum.tile([P, N // 2], F16, tag="finb")
        for t in range(G):
            dst = fin_a if t < 8 else fin_b
            nc.tensor.transpose(
                dst[:, P * (t % 8):P * (t % 8 + 1)],
                cm[:, P * t:P * (t + 1)], ident16)
        for q, fin_t in enumerate((fin_a, fin_b)):
            nc.vector.tensor_reduce(
                colmin[:, 8 * q:8 * (q + 1)],
                fin_t.rearrange("p (t c) -> p t c", c=P),
                axis=AX, op=OP.min)

        stot = sing.tile([P, 1], F32)
        nc.vector.tensor_reduce(stot, colmin, axis=AX, op=OP.add)
        ones = sing.tile([P, 1], F32)
        nc.vector.memset(ones, 1.0)

        tot = fin_psum.tile([1, 1], F32, tag="tot")
        nc.tensor.matmul(tot, stot, ones, start=True, stop=True)
        res = sing.tile([1, 1], F32)
        nc.scalar.activation(out=res, in_=tot, func=ACTF.Copy,
                             scale=0.5)
        nc.sync.dma_start(out=out_dram, in_=res)

    s_pool.release()
    work.release()
    sing.release()


def build_nc():
    nc = bacc.Bacc(trn_type="TRN2", target_bir_lowering=False, debug=False)
    mu_a = nc.dram_tensor("mu_a", [N, D], F32, kind="ExternalInput").ap()
    la_ = nc.dram_tensor("la", [N, D], F32, kind="ExternalInput").ap()
    mu_b = nc.dram_tensor("mu_b", [N, D], F32, kind="ExternalInput").ap()
    lb_ = nc.dram_tensor("lb", [N, D], F32, kind="ExternalInput").ap()
    out = nc.dram_tensor("out", [1, 1], F32, kind="ExternalOutput").ap()
    with tile.TileContext(nc) as tc:
        _chamfer_tile_kernel(tc, out, mu_a, la_, mu_b, lb_)
    nc.compile()
    return nc


_NC_CACHE = None


def _get_nc():
    global _NC_CACHE
    if _NC_CACHE is None:
        _NC_CACHE = build_nc()
    return _NC_CACHE


def _in_maps(mu_preds, logvar_preds, mu_gts, logvar_gts):
    maps = []
    for c in range(BS):
        maps.append({
            "mu_a": np.ascontiguousarray(mu_preds[c], dtype=np.float32),
            "la": np.ascontiguousarray(logvar_preds[c], dtype=np.float32),
            "mu_b": np.ascontiguousarray(mu_gts[c], dtype=np.float32),
            "lb": np.ascontiguousarray(logvar_gts[c], dtype=np.float32),
        })
    return maps


def run(mu_preds, logvar_preds, mu_gts, logvar_gts, trace=False):
    """Returns (out [8] float32, exec_time_ns or None)."""
    from concourse.bass_utils import run_bass_kernel_spmd
    nc = _get_nc()
    maps = _in_maps(mu_preds, logvar_preds, mu_gts, logvar_gts)
    r = run_bass_kernel_spmd(nc, maps, core_ids=list(range(BS)), trace=trace)
    out = np.array([r.results[c]["out"][0, 0] for c in range(BS)],
                   dtype=np.float32)
    return out, r.exec_time_ns


def kernel(mu_preds, logvar_preds, mu_gts, logvar_gts):
    out, _ = run(mu_preds, logvar_preds, mu_gts, logvar_gts, trace=False)
    return out


# revision 50
# speedup vs baseline: 102.4616x; 102.4616x over previous
"""Chamfer-KL loss kernel for Trainium2 (Bass/Tile).

Math: KL(N_i || N_j) summed over d for all pairs reduces to a rank-10
inner product.  With a = preds, b = gts, d = 4:

  KL[i,j] = 0.5 * (F_i . G_j)
  F_i = [exp(la_i)+mu_a_i^2 (4), -2*mu_a_i (4), 1, -sum_d la_i]
  G_j = [exp(-lb_j) (4), mu_b_j*exp(-lb_j) (4),
         sum_d mu_b_j^2*exp(-lb_j) + sum_d lb_j - 4, 1]

  out = 0.5 * (sum_j min_i (F_i.G_j)  +  sum_i min_j (F_i.G_j))

Sharding: data-parallel over batch, one batch element per NeuronCore
(bs=8 over 8 cores).  Per core the 2048x2048 pairwise matrix is produced
tile-by-tile by the TensorEngine (float32r matmuls, rank 10) into PSUM
and never hits HBM; mins are reduced flash-style on the fly:
  - ScalarE copies each PSUM tile to SBUF as fp16
  - VectorE computes the row-min as a fused fold+reduce over the fp16
    copy and keeps a running elementwise column-min (both lagged one
    tile so they never stall on ScalarE); column mins cross partitions
    at the end via 16 PE transposes + two free-axis reduces.
(GpSimd has no min/max ops and PSUM allows one read operand per
instruction, which rules out the cheaper-looking variants.)
"""

import numpy as np

import concourse.bacc as bacc
import concourse.bass as bass
import concourse.mybir as mybir
import concourse.tile as tile
from concourse.masks import make_identity

BS = 8          # batch size == number of cores
N = 2048        # points per cloud
D = 4           # point dimension
P = 128         # SBUF partitions
PT = N // P     # 16 points per partition in the raw layout
K = 2 * D + 2   # 10 live feature dims
NBLK = 512      # moving-operand columns per matmul (one PSUM bank fp32)
NB = N // NBLK  # 4 j-blocks per i-block
G = N // P      # 16 i-blocks

F32 = mybir.dt.float32
F32R = mybir.dt.float32r
F16 = mybir.dt.float16
AX = mybir.AxisListType.X
OP = mybir.AluOpType
ACTF = mybir.ActivationFunctionType


def _chamfer_tile_kernel(tc, out_dram, mu_a, la, mu_b, lb):
    nc = tc.nc

    sing = tc.alloc_tile_pool(name="sing", bufs=1)
    work = tc.alloc_tile_pool(name="work", bufs=1)
    s_pool = tc.alloc_tile_pool(name="s_pool", bufs=3)

    # Identities first: gpsimd is otherwise idle and the PE pre-warm
    # depends on ident16.
    ident32 = sing.tile([P, P], F32)
    make_identity(nc, ident32)
    ident16 = sing.tile([P, P], F16)
    make_identity(nc, ident16)

    # ---- load raw inputs: [2048, 4] -> [128, 16, 4] (row chunks) ----
    # Three DGE queues; G-side inputs (lb, mu_b) first since the G side
    # gates the first matmuls.
    t_ma = work.tile([P, PT, D], F32)
    t_la = work.tile([P, PT, D], F32)
    t_mb = work.tile([P, PT, D], F32)
    t_lb = work.tile([P, PT, D], F32)
    for (t, src), eng in zip(
            ((t_lb, lb), (t_mb, mu_b), (t_la, la), (t_ma, mu_a)),
            (nc.sync, nc.scalar, nc.gpsimd, nc.sync)):
        eng.dma_start(out=t, in_=src.rearrange("(p t) d -> p t d", p=P))

    # ---- PE pre-warm ----
    # The HAM clock gate keeps a cold PE at half rate for its first
    # ~3.4us; burn no-dep junk matmuls so the feature transposes and the
    # first real matmuls run at full clock.
    with tc.tile_pool(name="warm_psum", bufs=1, space="PSUM") as warm_psum:
        junk = warm_psum.tile([P, P], F32, tag="warm")
        for _ in range(12):
            nc.tensor.matmul(junk, ident16, ident16, start=True, stop=True)

    # ---- feature matrices in interleaved layout [128, 16, 10] ----
    # f128[p, t, k] = feature k of point (16*p + t)
    f128 = work.tile([P, PT, K], F32)
    g128 = work.tile([P, PT, K], F32)

    # G side first: its transposes + copies gate the first matmuls.
    # exp(-lb) is written strided straight into g128 by ACT; DVE reads it
    # back strided, saving two copies.
    nc.scalar.activation(out=g128[:, :, 0:D], in_=t_lb, func=ACTF.Exp,
                         scale=-1.0)
    nc.vector.tensor_mul(g128[:, :, D:2 * D], t_mb, g128[:, :, 0:D])
    t_q2 = work.tile([P, PT, D], F32)
    nc.vector.tensor_mul(t_q2, t_mb, g128[:, :, D:2 * D])
    t_r = work.tile([P, PT], F32)
    nc.vector.tensor_reduce(t_r, t_q2, axis=AX, op=OP.add)
    t_slb = work.tile([P, PT], F32)
    nc.vector.tensor_reduce(t_slb, t_lb, axis=AX, op=OP.add)
    # g128 k=8: (sum_d mub^2 ivb - 4) + sum_d lb, in one fused op
    nc.vector.scalar_tensor_tensor(
        out=g128[:, :, 2 * D], in0=t_r, scalar=-float(D), in1=t_slb,
        op0=OP.add, op1=OP.add)
    nc.vector.memset(g128[:, :, 2 * D + 1], 1.0)

    t_sq = work.tile([P, PT, D], F32)
    nc.vector.tensor_mul(t_sq, t_ma, t_ma)
    nc.scalar.activation(out=f128[:, :, 0:D], in_=t_la, func=ACTF.Exp)
    nc.vector.tensor_tensor(
        f128[:, :, 0:D], f128[:, :, 0:D], t_sq, OP.add)
    nc.vector.tensor_scalar_mul(f128[:, :, D:2 * D], t_ma, -2.0)
    nc.vector.memset(f128[:, :, 2 * D], 1.0)
    nc.vector.tensor_reduce(
        f128[:, :, 2 * D + 1], t_la, axis=AX, op=OP.add, negate=True)

    f128f = f128.rearrange("p t k -> p (t k)")
    g128f = g128.rearrange("p t k -> p (t k)")

    # ---- transpose features so k lands on partitions ----
    # Both sides become [10, 2048] (k on partitions 0..10, all points on
    # the free axis — matmul operands must share base partition 0).
    # G side is split into two [10, 1024] halves so the first matmuls can
    # start after half the copies.
    # Interleave G/F transpose batches with their PSUM->SBUF copies (gt
    # halves on ACT, ft halves on DVE) so copies overlap later transposes.
    with tc.tile_pool(name="pro_psum", bufs=1, space="PSUM") as pro_psum:
        p_gt_a = pro_psum.tile([K, N // 2], F32, tag="gta")
        p_gt_b = pro_psum.tile([K, N // 2], F32, tag="gtb")
        p_ft_a = pro_psum.tile([K, N // 2], F32, tag="fta")
        p_ft_b = pro_psum.tile([K, N // 2], F32, tag="ftb")
        gt_a = work.tile([K, N // 2], F32R)
        gt_b = work.tile([K, N // 2], F32R)
        ft_a = work.tile([K, N // 2], F32R)
        ft_b = work.tile([K, N // 2], F32R)

        def tr_batch(dst, srcf, lo):
            for h in range(lo, lo + 8):
                nc.tensor.transpose(
                    dst[:, P * (h % 8):P * (h % 8 + 1)],
                    srcf[:, K * h:K * (h + 1)], ident32)

        tr_batch(p_gt_a, g128f, 0)
        nc.scalar.copy(gt_a, p_gt_a)
        tr_batch(p_ft_a, f128f, 0)
        nc.vector.tensor_copy(ft_a, p_ft_a)
        tr_batch(p_gt_b, g128f, 8)
        nc.scalar.copy(gt_b, p_gt_b)
        tr_batch(p_ft_b, f128f, 8)
        nc.vector.tensor_copy(ft_b, p_ft_b)

    # ---- main loop: rank-10 matmuls + flash-style min reductions ----
    # (GpSimd supports no min/max ops, so all mins live on VectorE.)
    rm_all = sing.tile([P, G], F32)      # per-i row-min, one column per g
    cm = sing.tile([P, N], F16)          # running column-min

    def rm_update(g, sg):
        # Row-min via two elementwise fold halvings then a short reduce:
        # 2048 -> 1024 -> 512 -> 256 -> 1 per partition.  Plain TT/reduce ops
        # only (tensor_tensor_reduce dies on hardware).
        f1 = s_pool.tile([P, N // 2], F16, tag="fold", bufs=2, name="f1")
        nc.vector.tensor_tensor(f1, sg[:, 0:N // 2], sg[:, N // 2:N],
                                OP.min)
        f2 = s_pool.tile([P, N // 4], F16, tag="fold2", bufs=2, name="f2")
        nc.vector.tensor_tensor(f2, f1[:, 0:N // 4], f1[:, N // 4:N // 2],
                                OP.min)
        f3 = s_pool.tile([P, N // 8], F16, tag="fold3", bufs=2, name="f3")
        nc.vector.tensor_tensor(f3, f2[:, 0:N // 8], f2[:, N // 8:N // 4],
                                OP.min)
        nc.vector.tensor_reduce(
            rm_all[:, g:g + 1], f3, axis=AX, op=OP.min)

    def cm_update(g, sg):
        if g == 0:
            nc.vector.tensor_copy(cm, sg)
        else:
            nc.vector.tensor_tensor(cm, cm, sg, OP.min)

    with tc.tile_pool(name="mm_psum", bufs=2, space="PSUM") as mm_psum:
        sg_prev = None
        for g in range(G):
            pg = mm_psum.tile([P, N], F32, tag="mm")
            ft_t = ft_a if g < 8 else ft_b
            lhsT = ft_t[:, P * (g % 8):P * (g % 8 + 1)]
            for n in range(NB):
                rhs_t = gt_a if n < 2 else gt_b
                nc.tensor.matmul(
                    pg[:, NBLK * n:NBLK * (n + 1)],
                    lhsT,
                    rhs_t[:, NBLK * (n % 2):NBLK * (n % 2 + 1)],
                    start=True, stop=True)
            sg = s_pool.tile([P, N], F16, tag="s", bufs=5)
            nc.scalar.copy(sg, pg)
            # Row-min + column-min both lag one iteration so they consume
            # the previous, already-copied sg — no DVE stall on ACT.  The
            # row-min folds the two sg halves elementwise (throwaway out)
            # and reduces the fold per partition in one fused DVE op.
            if sg_prev is not None:
                rm_update(g - 1, sg_prev)
                cm_update(g - 1, sg_prev)
            if g == G - 1:
                # Final row-min un-lagged: it gates the row-sum in the
                # finalize chain and has no later copy to hide behind.
                rm_update(g, sg)
            sg_prev = sg
        # Epilogue: the last column-min update in four column chunks so
        # the finalize transposes start per-chunk.
        for c in range(4):
            lo, hi = (N // 4) * c, (N // 4) * (c + 1)
            nc.vector.tensor_tensor(
                cm[:, lo:hi], cm[:, lo:hi], sg_prev[:, lo:hi], OP.min)

    # ---- finalize ----
    # column mins: cross-partition min via 16 PE transposes, then four
    # free-axis reduces over [128, 4, 128] (split so they overlap the
    # transposes).
    with tc.tile_pool(name="fin_psum", bufs=1, space="PSUM") as fin_psum:
        # colmin has G+1 columns: 16 per-chunk column-mins plus the row-min
        # sum folded in as the 17th, so one reduce yields the grand total.
        colmin = sing.tile([P, G + 1], F32)
        nc.vector.tensor_reduce(
            colmin[:, G:G + 1], rm_all, axis=AX, op=OP.add)

        # fin in two tiles so the first reduce starts after 8 transposes
        # (readers of a tile wait on all of its writers).
        fin_a = fin_psum.tile([P, N // 2], F16, tag="fina")
        fin_b = fin_psum.tile([P, N // 2], F16, tag="finb")
        for t in range(G):
            dst = fin_a if t < 8 else fin_b
            nc.tensor.transpose(
                dst[:, P * (t % 8):P * (t % 8 + 1)],
                cm[:, P * t:P * (t + 1)], ident16)
        for q, fin_t in enumerate((fin_a, fin_b)):
            nc.vector.tensor_reduce(
                colmin[:, 8 * q:8 * (q + 1)],
                fin_t.rearrange("p (t c) -> p t c", c=P),
                axis=AX, op=OP.min)

        stot = sing.tile([P, 1], F32)
        nc.vector.tensor_reduce(stot, colmin, axis=AX, op=OP.add)
        ones = sing.tile([P, 1], F32)
        nc.vector.memset(ones, 1.0)

        tot = fin_psum.tile([1, 1], F32, tag="tot")
        nc.tensor.matmul(tot, stot, ones, start=True, stop=True)
        res = sing.tile([1, 1], F32)
        nc.scalar.activation(out=res, in_=tot, func=ACTF.Copy,
                             scale=0.5)
        nc.sync.dma_start(out=out_dram, in_=res)

    s_pool.release()
    work.release()
    sing.release()


def build_nc():
    nc = bacc.Bacc(trn_type="TRN2", target_bir_lowering=False, debug=False)
    mu_a = nc.dram_tensor("mu_a", [N, D], F32, kind="ExternalInput").ap()
    la_ = nc.dram_tensor("la", [N, D], F32, kind="ExternalInput").ap()
    mu_b = nc.dram_tensor("mu_b", [N, D], F32, kind="ExternalInput").ap()
    lb_ = nc.dram_tensor("lb", [N, D], F32, kind="ExternalInput").ap()
    out = nc.dram_tensor("out", [1, 1], F32, kind="ExternalOutput").ap()
    with tile.TileContext(nc) as tc:
        _chamfer_tile_kernel(tc, out, mu_a, la_, mu_b, lb_)
    nc.compile()
    return nc


_NC_CACHE = None


def _get_nc():
    global _NC_CACHE
    if _NC_CACHE is None:
        _NC_CACHE = build_nc()
    return _NC_CACHE


def _in_maps(mu_preds, logvar_preds, mu_gts, logvar_gts):
    maps = []
    for c in range(BS):
        maps.append({
            "mu_a": np.ascontiguousarray(mu_preds[c], dtype=np.float32),
            "la": np.ascontiguousarray(logvar_preds[c], dtype=np.float32),
            "mu_b": np.ascontiguousarray(mu_gts[c], dtype=np.float32),
            "lb": np.ascontiguousarray(logvar_gts[c], dtype=np.float32),
        })
    return maps


def run(mu_preds, logvar_preds, mu_gts, logvar_gts, trace=False):
    """Returns (out [8] float32, exec_time_ns or None)."""
    from concourse.bass_utils import run_bass_kernel_spmd
    nc = _get_nc()
    maps = _in_maps(mu_preds, logvar_preds, mu_gts, logvar_gts)
    r = run_bass_kernel_spmd(nc, maps, core_ids=list(range(BS)), trace=trace)
    out = np.array([r.results[c]["out"][0, 0] for c in range(BS)],
                   dtype=np.float32)
    return out, r.exec_time_ns


def kernel(mu_preds, logvar_preds, mu_gts, logvar_gts):
    out, _ = run(mu_preds, logvar_preds, mu_gts, logvar_gts, trace=False)
    return out
